# revision 57
# baseline (speedup 1.0000x reference)
import numpy as np

from concourse import bass, bacc, tile
from concourse.bass import broadcast_tensor_aps
from concourse.bass_utils import run_bass_kernel_spmd

B, S, D_IN, D_HID, D_OUT = 8, 4096, 512, 1024, 512
T = 512
NCHUNK = S // T
KG_IN = D_IN // 128    # 4
MG_HID = D_HID // 128  # 8
MG_OUT = D_OUT // 128  # 4

FP32 = bass.mybir.dt.float32
F32R = bass.mybir.dt.float32r
AF = bass.mybir.ActivationFunctionType
ALU = bass.mybir.AluOpType


def build_nc():
    nc = bacc.Bacc(None, target_bir_lowering=False)

    xT = nc.dram_tensor("xT", [NCHUNK, D_IN, T], FP32, kind="ExternalInput")
    a_sm = nc.dram_tensor("a_sm", [128, MG_HID], FP32, kind="ExternalInput")
    h0T = nc.dram_tensor("h0T", [128, MG_HID], FP32, kind="ExternalInput")
    w_in = nc.dram_tensor("w_in", [D_IN, D_HID], FP32, kind="ExternalInput")
    w_dx = nc.dram_tensor("w_dx", [D_IN, D_OUT], FP32, kind="ExternalInput")
    w_out = nc.dram_tensor("w_out", [D_HID, D_OUT], FP32, kind="ExternalInput")
    b_inT = nc.dram_tensor("b_inT", [128, MG_HID], FP32, kind="ExternalInput")
    b_oT = nc.dram_tensor("b_oT", [128, MG_OUT], FP32, kind="ExternalInput")
    outT = nc.dram_tensor("outT", [NCHUNK, D_OUT, T], FP32, kind="ExternalOutput")
    h_lastT = nc.dram_tensor("h_lastT", [128, MG_HID], F32R, kind="ExternalOutput")

    with tile.TileContext(nc) as tc:
        with (
            tc.tile_pool(name="wp", bufs=1) as wp,
            tc.tile_pool(name="xp", bufs=3) as xp,
            tc.tile_pool(name="up", bufs=2) as up,
            tc.tile_pool(name="hp", bufs=2) as hp,
            tc.tile_pool(name="op", bufs=2) as op,
            tc.tile_pool(name="ups", bufs=4, space="PSUM") as ups,
            tc.tile_pool(name="ops", bufs=4, space="PSUM") as ops,
        ):
            w_in_t = wp.tile([128, KG_IN, MG_HID, 128], F32R)
            w_dx_t = wp.tile([128, KG_IN, MG_OUT, 128], F32R)
            w_out_t = wp.tile([128, MG_HID, MG_OUT, 128], F32R)
            a_sm_t = wp.tile([128, MG_HID], FP32)
            h0_t = wp.tile([128, MG_HID], FP32)
            b_in_t = wp.tile([128, MG_HID], FP32)
            b_o_t = wp.tile([128, MG_OUT], FP32)

            def load_x(c):
                x_t = xp.tile([128, KG_IN, T], F32R, tag="x")
                for kg in range(KG_IN):
                    nc.sync.dma_start(
                        x_t[:, kg],
                        xT[c, kg * 128:(kg + 1) * 128, :].bitcast(F32R))
                return x_t

            x0_t = xp.tile([128, KG_IN, T], F32R, tag="x")

            # PE warm-up: dummy matmuls on a scratch tile keep the tensor
            # engine active during the startup DMA window so the first real
            # matmuls run at steady cadence instead of ~1.8x slow.
            scr = wp.tile([128, T], FP32)
            nc.vector.memset(scr, 1.0)
            scr_r = scr.bitcast(F32R)
            psd = ups.tile([128, T], FP32, tag="ups", name="warm")
            for _ in range(11):
                nc.tensor.matmul(psd, scr_r[:, 0:128], scr_r,
                                 start=True, stop=True)

            def emit_u(x_t):
                u_sb = up.tile([128, MG_HID, T], FP32, tag="u")
                for mg in range(MG_HID):
                    ps = ups.tile([128, T], FP32, tag="ups")
                    for kg in range(KG_IN):
                        nc.tensor.matmul(ps, w_in_t[:, kg, mg], x_t[:, kg],
                                         start=(kg == 0), stop=(kg == KG_IN - 1))
                    nc.scalar.activation(u_sb[:, mg], ps, AF.Identity,
                                         bias=b_in_t[:, mg:mg + 1], scale=1.0)
                return u_sb

            # Startup DMAs, all on the sync HWDGE queue, kg-interleaved so
            # chunk-0's kg0-2 accumulations start as the slices land.
            for kg in range(KG_IN):
                nc.sync.dma_start(
                    w_in_t[:, kg], w_in[kg * 128:(kg + 1) * 128, :].bitcast(F32R))
                nc.sync.dma_start(
                    x0_t[:, kg],
                    xT[0, kg * 128:(kg + 1) * 128, :].bitcast(F32R))
            nc.sync.dma_start(b_in_t, b_inT[:, :])
            nc.sync.dma_start(a_sm_t, a_sm[:, :])
            nc.sync.dma_start(h0_t, h0T[:, :])
            x1_t = load_x(1)
            for kg in range(KG_IN):
                nc.sync.dma_start(
                    w_dx_t[:, kg], w_dx[kg * 128:(kg + 1) * 128, :].bitcast(F32R))
            for kg in range(MG_HID):
                nc.sync.dma_start(
                    w_out_t[:, kg], w_out[kg * 128:(kg + 1) * 128, :].bitcast(F32R))
            nc.sync.dma_start(b_o_t, b_oT[:, :])

            # Chunk-0 u across all 8 PSUM banks (borrowing the ops pool):
            # every mg's kg0-2 accumulation runs while the startup DMA is
            # still streaming, leaving only the 8 kg3 finishers for after the
            # last w_in/x0 slice lands.
            x_next = x1_t
            u_cur = up.tile([128, MG_HID, T], FP32, tag="u")
            pss = [(ups if m < 4 else ops).tile(
                       [128, T], FP32, tag=("ups" if m < 4 else "ops"),
                       name=f"u0p{m}") for m in range(MG_HID)]
            for mg in range(MG_HID):
                for kg in range(3):
                    nc.tensor.matmul(pss[mg], w_in_t[:, kg, mg], x0_t[:, kg],
                                     start=(kg == 0), stop=False)
            for mg in range(MG_HID):
                nc.tensor.matmul(pss[mg], w_in_t[:, 3, mg], x0_t[:, 3],
                                 start=False, stop=True)
            for mg in range(MG_HID):
                nc.scalar.activation(u_cur[:, mg], pss[mg], AF.Identity,
                                     bias=b_in_t[:, mg:mg + 1], scale=1.0)
            x_cur = x0_t
            h_prev = None
            for c in range(NCHUNK):
                x_t, u_sb = x_cur, u_cur

                # Prefetch x two chunks ahead so its DMA has a full chunk of
                # lead time and never stalls the next chunk's u-matmuls.
                if c + 2 < NCHUNK:
                    x_pref = load_x(c + 2)

                h_sb = hp.tile([128, MG_HID, T], F32R, tag="h")
                for mg in range(MG_HID):
                    init = h0_t[:, mg:mg + 1] if c == 0 else h_prev[:, mg, T - 1:T]
                    a_ap, u_ap = broadcast_tensor_aps(
                        a_sm_t[:, mg:mg + 1], u_sb[:, mg])
                    nc.vector.tensor_tensor_scan(
                        h_sb[:, mg], a_ap, u_ap, init,
                        op0=ALU.mult, op1=ALU.add)
                h_prev = h_sb
                if c == NCHUNK - 1:
                    nc.sync.dma_start(h_lastT[:, :], h_sb[:, :, T - 1])

                # Pipeline: next chunk's u-matmuls run on PE while DVE scans
                # this chunk, ahead of out-matmuls that depend on the scan.
                if c + 1 < NCHUNK:
                    u_cur = emit_u(x_next)
                    x_cur = x_next
                if c + 2 < NCHUNK:
                    x_next = x_pref

                out_sb = op.tile([128, MG_OUT, T], FP32, tag="o")
                for mg in range(MG_OUT):
                    ps = ops.tile([128, T], FP32, tag="ops")
                    for kg in range(KG_IN):
                        nc.tensor.matmul(ps, w_dx_t[:, kg, mg], x_t[:, kg],
                                         start=(kg == 0), stop=False)
                    for kg in range(MG_HID):
                        nc.tensor.matmul(ps, w_out_t[:, kg, mg], h_sb[:, kg],
                                         start=False, stop=(kg == MG_HID - 1))
                    nc.scalar.activation(out_sb[:, mg], ps, AF.Identity,
                                         bias=b_o_t[:, mg:mg + 1], scale=0.5)
                for mg in range(MG_OUT):
                    nc.sync.dma_start(
                        outT[c, mg * 128:(mg + 1) * 128, :], out_sb[:, mg])
    nc.finalize()
    return nc


def prepare_in_maps(x, h0, a_logit, W_dx, b_dx, W_in, b_in, W_out, b_out):
    a = (1.0 / (1.0 + np.exp(-a_logit.astype(np.float64)))).astype(np.float32)
    g = np.sqrt(np.float32(1.0) - a * a)

    w_in_s = np.ascontiguousarray(W_in * g[None, :])
    b_in_s = b_in * g
    b_o = (b_dx + b_out) * np.float32(0.5)

    aT = np.ascontiguousarray(a.reshape(MG_HID, 128).T).astype(np.float32)
    b_inT = np.ascontiguousarray(b_in_s.reshape(MG_HID, 128).T)
    b_oT = np.ascontiguousarray(b_o.reshape(MG_OUT, 128).T)

    in_maps = []
    for b in range(B):
        in_maps.append({
            "xT": np.ascontiguousarray(
                x[b].T.reshape(D_IN, NCHUNK, T).transpose(1, 0, 2)),
            "a_sm": aT,
            "h0T": np.ascontiguousarray(h0[b].reshape(MG_HID, 128).T),
            "w_in": w_in_s,
            "w_dx": np.ascontiguousarray(W_dx),
            "w_out": np.ascontiguousarray(W_out),
            "b_inT": b_inT,
            "b_oT": b_oT,
        })
    return in_maps


def postprocess(results):
    out = np.empty((B, S, D_OUT), np.float32)
    h_last = np.empty((B, D_HID), np.float32)
    for b in range(B):
        out[b] = results[b]["outT"].transpose(0, 2, 1).reshape(S, D_OUT)
        h_last[b] = results[b]["h_lastT"].T.reshape(D_HID)
    return out, h_last


def kernel(**inputs):
    nc = build_nc()
    in_maps = prepare_in_maps(**inputs)
    res = run_bass_kernel_spmd(nc, in_maps, core_ids=list(range(B)))
    return postprocess(res.results)


# revision 58
# speedup vs baseline: 1.0047x; 1.0047x over previous
import numpy as np

from concourse import bass, bacc, tile
from concourse.bass import broadcast_tensor_aps
from concourse.bass_utils import run_bass_kernel_spmd

B, S, D_IN, D_HID, D_OUT = 8, 4096, 512, 1024, 512
T = 512
NCHUNK = S // T
KG_IN = D_IN // 128    # 4
MG_HID = D_HID // 128  # 8
MG_OUT = D_OUT // 128  # 4

FP32 = bass.mybir.dt.float32
F32R = bass.mybir.dt.float32r
AF = bass.mybir.ActivationFunctionType
ALU = bass.mybir.AluOpType


def build_nc():
    nc = bacc.Bacc(None, target_bir_lowering=False)

    xT = nc.dram_tensor("xT", [NCHUNK, D_IN, T], FP32, kind="ExternalInput")
    a_sm = nc.dram_tensor("a_sm", [128, MG_HID], FP32, kind="ExternalInput")
    h0T = nc.dram_tensor("h0T", [128, MG_HID], FP32, kind="ExternalInput")
    w_in = nc.dram_tensor("w_in", [D_IN, D_HID], FP32, kind="ExternalInput")
    w_dx = nc.dram_tensor("w_dx", [D_IN, D_OUT], FP32, kind="ExternalInput")
    w_out = nc.dram_tensor("w_out", [D_HID, D_OUT], FP32, kind="ExternalInput")
    b_inT = nc.dram_tensor("b_inT", [128, MG_HID], FP32, kind="ExternalInput")
    b_oT = nc.dram_tensor("b_oT", [128, MG_OUT], FP32, kind="ExternalInput")
    outT = nc.dram_tensor("outT", [NCHUNK, D_OUT, T], FP32, kind="ExternalOutput")
    h_lastT = nc.dram_tensor("h_lastT", [128, MG_HID], F32R, kind="ExternalOutput")

    with tile.TileContext(nc) as tc:
        with (
            tc.tile_pool(name="wp", bufs=1) as wp,
            tc.tile_pool(name="xp", bufs=3) as xp,
            tc.tile_pool(name="up", bufs=2) as up,
            tc.tile_pool(name="hp", bufs=2) as hp,
            tc.tile_pool(name="op", bufs=2) as op,
            tc.tile_pool(name="ups", bufs=4, space="PSUM") as ups,
            tc.tile_pool(name="ops", bufs=4, space="PSUM") as ops,
        ):
            w_in_t = wp.tile([128, KG_IN, MG_HID, 128], F32R)
            w_dx_t = wp.tile([128, KG_IN, MG_OUT, 128], F32R)
            w_out_t = wp.tile([128, MG_HID, MG_OUT, 128], F32R)
            a_sm_t = wp.tile([128, MG_HID], FP32)
            h0_t = wp.tile([128, MG_HID], FP32)
            b_in_t = wp.tile([128, MG_HID], FP32)
            b_o_t = wp.tile([128, MG_OUT], FP32)

            def load_x(c):
                x_t = xp.tile([128, KG_IN, T], F32R, tag="x")
                for kg in range(KG_IN):
                    nc.sync.dma_start(
                        x_t[:, kg],
                        xT[c, kg * 128:(kg + 1) * 128, :].bitcast(F32R))
                return x_t

            x0_t = xp.tile([128, KG_IN, T], F32R, tag="x")

            # PE warm-up: dummy matmuls on a scratch tile keep the tensor
            # engine active during the startup DMA window so the first real
            # matmuls run at steady cadence instead of ~1.8x slow.
            scr = wp.tile([128, T], FP32)
            nc.vector.memset(scr, 1.0)
            scr_r = scr.bitcast(F32R)
            psd = ups.tile([128, T], FP32, tag="ups", name="warm")
            for _ in range(14):
                nc.tensor.matmul(psd, scr_r[:, 0:128], scr_r,
                                 start=True, stop=True)

            def emit_u(x_t):
                u_sb = up.tile([128, MG_HID, T], FP32, tag="u")
                for mg in range(MG_HID):
                    ps = ups.tile([128, T], FP32, tag="ups")
                    for kg in range(KG_IN):
                        nc.tensor.matmul(ps, w_in_t[:, kg, mg], x_t[:, kg],
                                         start=(kg == 0), stop=(kg == KG_IN - 1))
                    nc.scalar.activation(u_sb[:, mg], ps, AF.Identity,
                                         bias=b_in_t[:, mg:mg + 1], scale=1.0)
                return u_sb

            # Startup DMAs, all on the sync HWDGE queue, kg-interleaved so
            # chunk-0's kg0-2 accumulations start as the slices land.
            for kg in range(KG_IN):
                nc.sync.dma_start(
                    w_in_t[:, kg], w_in[kg * 128:(kg + 1) * 128, :].bitcast(F32R))
                nc.sync.dma_start(
                    x0_t[:, kg],
                    xT[0, kg * 128:(kg + 1) * 128, :].bitcast(F32R))
            nc.sync.dma_start(b_in_t, b_inT[:, :])
            nc.sync.dma_start(a_sm_t, a_sm[:, :])
            nc.sync.dma_start(h0_t, h0T[:, :])
            x1_t = load_x(1)
            for kg in range(KG_IN):
                nc.sync.dma_start(
                    w_dx_t[:, kg], w_dx[kg * 128:(kg + 1) * 128, :].bitcast(F32R))
            for kg in range(MG_HID):
                nc.sync.dma_start(
                    w_out_t[:, kg], w_out[kg * 128:(kg + 1) * 128, :].bitcast(F32R))
            nc.sync.dma_start(b_o_t, b_oT[:, :])

            # Chunk-0 u across all 8 PSUM banks (borrowing the ops pool):
            # every mg's kg0-2 accumulation runs while the startup DMA is
            # still streaming, leaving only the 8 kg3 finishers for after the
            # last w_in/x0 slice lands.
            x_next = x1_t
            u_cur = up.tile([128, MG_HID, T], FP32, tag="u")
            pss = [(ups if m < 4 else ops).tile(
                       [128, T], FP32, tag=("ups" if m < 4 else "ops"),
                       name=f"u0p{m}") for m in range(MG_HID)]
            for mg in range(MG_HID):
                for kg in range(3):
                    nc.tensor.matmul(pss[mg], w_in_t[:, kg, mg], x0_t[:, kg],
                                     start=(kg == 0), stop=False)
            for mg in range(MG_HID):
                nc.tensor.matmul(pss[mg], w_in_t[:, 3, mg], x0_t[:, 3],
                                 start=False, stop=True)
            for mg in range(MG_HID):
                nc.scalar.activation(u_cur[:, mg], pss[mg], AF.Identity,
                                     bias=b_in_t[:, mg:mg + 1], scale=1.0)
            x_cur = x0_t
            h_prev = None
            for c in range(NCHUNK):
                x_t, u_sb = x_cur, u_cur

                # Prefetch x two chunks ahead so its DMA has a full chunk of
                # lead time and never stalls the next chunk's u-matmuls.
                if c + 2 < NCHUNK:
                    x_pref = load_x(c + 2)

                h_sb = hp.tile([128, MG_HID, T], F32R, tag="h")
                for mg in range(MG_HID):
                    init = h0_t[:, mg:mg + 1] if c == 0 else h_prev[:, mg, T - 1:T]
                    a_ap, u_ap = broadcast_tensor_aps(
                        a_sm_t[:, mg:mg + 1], u_sb[:, mg])
                    nc.vector.tensor_tensor_scan(
                        h_sb[:, mg], a_ap, u_ap, init,
                        op0=ALU.mult, op1=ALU.add)
                h_prev = h_sb
                if c == NCHUNK - 1:
                    nc.sync.dma_start(h_lastT[:, :], h_sb[:, :, T - 1])

                # Pipeline: next chunk's u-matmuls run on PE while DVE scans
                # this chunk, ahead of out-matmuls that depend on the scan.
                if c + 1 < NCHUNK:
                    u_cur = emit_u(x_next)
                    x_cur = x_next
                if c + 2 < NCHUNK:
                    x_next = x_pref

                out_sb = op.tile([128, MG_OUT, T], FP32, tag="o")
                for mg in range(MG_OUT):
                    ps = ops.tile([128, T], FP32, tag="ops")
                    for kg in range(KG_IN):
                        nc.tensor.matmul(ps, w_dx_t[:, kg, mg], x_t[:, kg],
                                         start=(kg == 0), stop=False)
                    for kg in range(MG_HID):
                        nc.tensor.matmul(ps, w_out_t[:, kg, mg], h_sb[:, kg],
                                         start=False, stop=(kg == MG_HID - 1))
                    nc.scalar.activation(out_sb[:, mg], ps, AF.Identity,
                                         bias=b_o_t[:, mg:mg + 1], scale=0.5)
                for mg in range(MG_OUT):
                    nc.sync.dma_start(
                        outT[c, mg * 128:(mg + 1) * 128, :], out_sb[:, mg])
    nc.finalize()
    return nc


def prepare_in_maps(x, h0, a_logit, W_dx, b_dx, W_in, b_in, W_out, b_out):
    a = (1.0 / (1.0 + np.exp(-a_logit.astype(np.float64)))).astype(np.float32)
    g = np.sqrt(np.float32(1.0) - a * a)

    w_in_s = np.ascontiguousarray(W_in * g[None, :])
    b_in_s = b_in * g
    b_o = (b_dx + b_out) * np.float32(0.5)

    aT = np.ascontiguousarray(a.reshape(MG_HID, 128).T).astype(np.float32)
    b_inT = np.ascontiguousarray(b_in_s.reshape(MG_HID, 128).T)
    b_oT = np.ascontiguousarray(b_o.reshape(MG_OUT, 128).T)

    in_maps = []
    for b in range(B):
        in_maps.append({
            "xT": np.ascontiguousarray(
                x[b].T.reshape(D_IN, NCHUNK, T).transpose(1, 0, 2)),
            "a_sm": aT,
            "h0T": np.ascontiguousarray(h0[b].reshape(MG_HID, 128).T),
            "w_in": w_in_s,
            "w_dx": np.ascontiguousarray(W_dx),
            "w_out": np.ascontiguousarray(W_out),
            "b_inT": b_inT,
            "b_oT": b_oT,
        })
    return in_maps


def postprocess(results):
    out = np.empty((B, S, D_OUT), np.float32)
    h_last = np.empty((B, D_HID), np.float32)
    for b in range(B):
        out[b] = results[b]["outT"].transpose(0, 2, 1).reshape(S, D_OUT)
        h_last[b] = results[b]["h_lastT"].T.reshape(D_HID)
    return out, h_last


def kernel(**inputs):
    nc = build_nc()
    in_maps = prepare_in_maps(**inputs)
    res = run_bass_kernel_spmd(nc, in_maps, core_ids=list(range(B)))
    return postprocess(res.results)


# revision 59
# speedup vs baseline: 1.0150x; 1.0102x over previous
import numpy as np

from concourse import bass, bacc, tile
from concourse.bass import broadcast_tensor_aps
from concourse.bass_utils import run_bass_kernel_spmd

B, S, D_IN, D_HID, D_OUT = 8, 4096, 512, 1024, 512
T = 512
NCHUNK = S // T
KG_IN = D_IN // 128    # 4
MG_HID = D_HID // 128  # 8
MG_OUT = D_OUT // 128  # 4

FP32 = bass.mybir.dt.float32
F32R = bass.mybir.dt.float32r
AF = bass.mybir.ActivationFunctionType
ALU = bass.mybir.AluOpType


def build_nc():
    nc = bacc.Bacc(None, target_bir_lowering=False)

    xT = nc.dram_tensor("xT", [NCHUNK, D_IN, T], FP32, kind="ExternalInput")
    a_sm = nc.dram_tensor("a_sm", [128, MG_HID], FP32, kind="ExternalInput")
    h0T = nc.dram_tensor("h0T", [128, MG_HID], FP32, kind="ExternalInput")
    w_in = nc.dram_tensor("w_in", [D_IN, D_HID], FP32, kind="ExternalInput")
    w_dx = nc.dram_tensor("w_dx", [D_IN, D_OUT], FP32, kind="ExternalInput")
    w_out = nc.dram_tensor("w_out", [D_HID, D_OUT], FP32, kind="ExternalInput")
    b_inT = nc.dram_tensor("b_inT", [128, MG_HID], FP32, kind="ExternalInput")
    b_oT = nc.dram_tensor("b_oT", [128, MG_OUT], FP32, kind="ExternalInput")
    outT = nc.dram_tensor("outT", [NCHUNK, D_OUT, T], FP32, kind="ExternalOutput")
    h_lastT = nc.dram_tensor("h_lastT", [128, MG_HID], F32R, kind="ExternalOutput")

    with tile.TileContext(nc) as tc:
        with (
            tc.tile_pool(name="wp", bufs=1) as wp,
            tc.tile_pool(name="xp", bufs=3) as xp,
            tc.tile_pool(name="up", bufs=2) as up,
            tc.tile_pool(name="hp", bufs=2) as hp,
            tc.tile_pool(name="op", bufs=2) as op,
            tc.tile_pool(name="ups", bufs=4, space="PSUM") as ups,
            tc.tile_pool(name="ops", bufs=4, space="PSUM") as ops,
        ):
            w_in_t = wp.tile([128, KG_IN, MG_HID, 128], F32R)
            w_dx_t = wp.tile([128, KG_IN, MG_OUT, 128], F32R)
            w_out_t = wp.tile([128, MG_HID, MG_OUT, 128], F32R)
            a_sm_t = wp.tile([128, MG_HID], FP32)
            h0_t = wp.tile([128, MG_HID], FP32)
            b_in_t = wp.tile([128, MG_HID], FP32)
            b_o_t = wp.tile([128, MG_OUT], FP32)

            def load_x(c):
                x_t = xp.tile([128, KG_IN, T], F32R, tag="x")
                for kg in range(KG_IN):
                    nc.sync.dma_start(
                        x_t[:, kg],
                        xT[c, kg * 128:(kg + 1) * 128, :].bitcast(F32R))
                return x_t

            x0_t = xp.tile([128, KG_IN, T], F32R, tag="x")

            # PE warm-up: dummy matmuls on a scratch tile keep the tensor
            # engine active during the startup DMA window so the first real
            # matmuls run at steady cadence instead of ~1.8x slow.
            scr = wp.tile([128, T], FP32)
            nc.vector.memset(scr, 1.0)
            scr_r = scr.bitcast(F32R)
            psd = ups.tile([128, T], FP32, tag="ups", name="warm")
            for _ in range(14):
                nc.tensor.matmul(psd, scr_r[:, 0:128], scr_r,
                                 start=True, stop=True)

            def emit_u(x_t):
                u_sb = up.tile([128, MG_HID, T], FP32, tag="u")
                for mg in range(MG_HID):
                    ps = ups.tile([128, T], FP32, tag="ups")
                    for kg in range(KG_IN):
                        nc.tensor.matmul(ps, w_in_t[:, kg, mg], x_t[:, kg],
                                         start=(kg == 0), stop=(kg == KG_IN - 1))
                    nc.scalar.activation(u_sb[:, mg], ps, AF.Identity,
                                         bias=b_in_t[:, mg:mg + 1], scale=1.0)
                return u_sb

            # Startup DMAs, all on the sync HWDGE queue, kg-interleaved so
            # chunk-0's kg0-2 accumulations start as the slices land.
            for kg in range(KG_IN):
                nc.sync.dma_start(
                    w_in_t[:, kg], w_in[kg * 128:(kg + 1) * 128, :].bitcast(F32R))
                nc.sync.dma_start(
                    x0_t[:, kg],
                    xT[0, kg * 128:(kg + 1) * 128, :].bitcast(F32R))
            nc.sync.dma_start(b_in_t, b_inT[:, :])
            nc.sync.dma_start(a_sm_t, a_sm[:, :])
            nc.sync.dma_start(h0_t, h0T[:, :])
            x1_t = load_x(1)
            for kg in range(KG_IN):
                nc.sync.dma_start(
                    w_dx_t[:, kg], w_dx[kg * 128:(kg + 1) * 128, :].bitcast(F32R))
            for kg in range(MG_HID):
                nc.sync.dma_start(
                    w_out_t[:, kg], w_out[kg * 128:(kg + 1) * 128, :].bitcast(F32R))
            nc.sync.dma_start(b_o_t, b_oT[:, :])

            # Chunk-0 u across all 8 PSUM banks (borrowing the ops pool):
            # every mg's kg0-2 accumulation runs while the startup DMA is
            # still streaming, leaving only the 8 kg3 finishers for after the
            # last w_in/x0 slice lands.
            x_next = x1_t
            u_cur = up.tile([128, MG_HID, T], FP32, tag="u")
            pss = [(ups if m < 4 else ops).tile(
                       [128, T], FP32, tag=("ups" if m < 4 else "ops"),
                       name=f"u0p{m}") for m in range(MG_HID)]
            for mg in range(MG_HID):
                for kg in range(3):
                    nc.tensor.matmul(pss[mg], w_in_t[:, kg, mg], x0_t[:, kg],
                                     start=(kg == 0), stop=False)
            for mg in range(MG_HID):
                nc.tensor.matmul(pss[mg], w_in_t[:, 3, mg], x0_t[:, 3],
                                 start=False, stop=True)
            for mg in range(MG_HID):
                nc.scalar.activation(u_cur[:, mg], pss[mg], AF.Identity,
                                     bias=b_in_t[:, mg:mg + 1], scale=1.0)
            x_cur = x0_t
            h_prev = None
            for c in range(NCHUNK):
                x_t, u_sb = x_cur, u_cur

                # Prefetch x two chunks ahead so its DMA has a full chunk of
                # lead time and never stalls the next chunk's u-matmuls.
                if c + 2 < NCHUNK:
                    x_pref = load_x(c + 2)

                h_sb = hp.tile([128, MG_HID, T], F32R, tag="h")
                for mg in range(MG_HID):
                    init = h0_t[:, mg:mg + 1] if c == 0 else h_prev[:, mg, T - 1:T]
                    a_ap, u_ap = broadcast_tensor_aps(
                        a_sm_t[:, mg:mg + 1], u_sb[:, mg])
                    nc.vector.tensor_tensor_scan(
                        h_sb[:, mg], a_ap, u_ap, init,
                        op0=ALU.mult, op1=ALU.add)
                h_prev = h_sb
                if c == NCHUNK - 1:
                    nc.sync.dma_start(h_lastT[:, :], h_sb[:, :, T - 1])

                # Pipeline: next chunk's u-matmuls run on PE while DVE scans
                # this chunk, ahead of out-matmuls that depend on the scan.
                if c + 1 < NCHUNK:
                    u_cur = emit_u(x_next)
                    x_cur = x_next
                if c + 2 < NCHUNK:
                    x_next = x_pref

                out_sb = op.tile([128, MG_OUT, T], FP32, tag="o")
                last = c == NCHUNK - 1
                for mg in range(MG_OUT):
                    ps = ops.tile([128, T], FP32, tag="ops")
                    for kg in range(KG_IN):
                        nc.tensor.matmul(ps, w_dx_t[:, kg, mg], x_t[:, kg],
                                         start=(kg == 0), stop=False)
                    for kg in range(MG_HID):
                        nc.tensor.matmul(ps, w_out_t[:, kg, mg], h_sb[:, kg],
                                         start=False, stop=(kg == MG_HID - 1))
                    if last and mg == MG_OUT - 1:
                        # Halve only the ACT+store of the very last group so
                        # the two stores pipeline their DMA latency.
                        for th in range(2):
                            sl = slice(th * 256, th * 256 + 256)
                            nc.scalar.activation(out_sb[:, mg, sl], ps[:, sl],
                                                 AF.Identity,
                                                 bias=b_o_t[:, mg:mg + 1],
                                                 scale=0.5)
                            nc.sync.dma_start(
                                outT[c, mg * 128:(mg + 1) * 128, sl],
                                out_sb[:, mg, sl])
                    else:
                        nc.scalar.activation(out_sb[:, mg], ps, AF.Identity,
                                             bias=b_o_t[:, mg:mg + 1],
                                             scale=0.5)
                for mg in range(MG_OUT - 1 if last else MG_OUT):
                    nc.sync.dma_start(
                        outT[c, mg * 128:(mg + 1) * 128, :], out_sb[:, mg])
    nc.finalize()
    return nc


def prepare_in_maps(x, h0, a_logit, W_dx, b_dx, W_in, b_in, W_out, b_out):
    a = (1.0 / (1.0 + np.exp(-a_logit.astype(np.float64)))).astype(np.float32)
    g = np.sqrt(np.float32(1.0) - a * a)

    w_in_s = np.ascontiguousarray(W_in * g[None, :])
    b_in_s = b_in * g
    b_o = (b_dx + b_out) * np.float32(0.5)

    aT = np.ascontiguousarray(a.reshape(MG_HID, 128).T).astype(np.float32)
    b_inT = np.ascontiguousarray(b_in_s.reshape(MG_HID, 128).T)
    b_oT = np.ascontiguousarray(b_o.reshape(MG_OUT, 128).T)

    in_maps = []
    for b in range(B):
        in_maps.append({
            "xT": np.ascontiguousarray(
                x[b].T.reshape(D_IN, NCHUNK, T).transpose(1, 0, 2)),
            "a_sm": aT,
            "h0T": np.ascontiguousarray(h0[b].reshape(MG_HID, 128).T),
            "w_in": w_in_s,
            "w_dx": np.ascontiguousarray(W_dx),
            "w_out": np.ascontiguousarray(W_out),
            "b_inT": b_inT,
            "b_oT": b_oT,
        })
    return in_maps


def postprocess(results):
    out = np.empty((B, S, D_OUT), np.float32)
    h_last = np.empty((B, D_HID), np.float32)
    for b in range(B):
        out[b] = results[b]["outT"].transpose(0, 2, 1).reshape(S, D_OUT)
        h_last[b] = results[b]["h_lastT"].T.reshape(D_HID)
    return out, h_last


def kernel(**inputs):
    nc = build_nc()
    in_maps = prepare_in_maps(**inputs)
    res = run_bass_kernel_spmd(nc, in_maps, core_ids=list(range(B)))
    return postprocess(res.results)
